# revision 4
# baseline (speedup 1.0000x reference)
"""Trainium2 Bass kernel for nn_GatedJunction (gated multi-branch junction).

Math (per batch element b):
    m_y  = mean_hw(y[b])                     # [C]
    m_xk = mean_hw(x_k[b])                   # [C] for k=0..3
    feats = concat(m_y, m_x0..m_x3)          # [5C] = [1280]
    h  = relu(bn(feats @ conv1_w.T))         # [32]
    w  = h @ conv2_w.T + conv2_b             # [1280] -> [5, 256]
    w1 = sigmoid(w[0])                       # self gate  [256]
    w2 = softmax_k(w[1:])                    # branch gates [4, 256]
    out[b] = y[b]*w1[:,None,None] + sum_k w2[k][:,None,None]*x_k[b]

Sharding: data-parallel over batch. 8 cores x 4 batch elements each.
Params are tiny, pre-transposed/folded on the host, replicated per core.

This version halves HBM traffic vs fp32 by moving the 5 big input maps and
the output to fp16 (quantization error ~1e-3 rel, well inside tolerance):
  - inputs are cast to fp16 on the host; tiles are [128, 2, 1024] fp16
  - channel sums for the gate MLP: DVE tensor_scalar(accum_out=...) which
    runs in the 4x fp16 perf mode
  - gate MLP runs on PE in fp32 (tiny), gates land channel-on-partition
  - pass 2 runs on PE as diag(gate_k) @ x_k matmuls (fp16, 1 cyc/row)
    accumulated in PSUM; the final merge out = y*w1 + psum is a
    scalar_tensor_tensor on DVE/Pool that downcasts to fp16 for the store
"""

import sys

for _p in ("/root/.axon_site/_ro/trn_rl_repo", "/opt/trn_rl_repo"):
    if _p not in sys.path:
        sys.path.append(_p)

from contextlib import ExitStack

import numpy as np

import concourse.bass as bass
import concourse.tile as tile
from concourse import mybir
from concourse.bass_utils import run_bass_kernel_spmd

# Problem constants (hardcoded from the spec).
B, K, C, H, W = 32, 4, 256, 32, 32
MID = 32
EPS = 1e-5
HW = H * W          # 1024
HWH = HW // 2       # 512 (one PSUM bank of fp32)
N_CORES = 8
B_LOC = B // N_CORES  # 4
NT = K + 1          # 5 tensors: y, x0..x3
FEAT = NT * C       # 1280
NCH = FEAT // 128   # 10 feature chunks of 128
CH = C // 128       # 2 channel chunks per tensor

FP32 = mybir.dt.float32
FP16 = mybir.dt.float16
ALU = mybir.AluOpType
AF = mybir.ActivationFunctionType


def _split_waits(nc: bass.Bass) -> None:
    """This toolchain's walrus accepts only ONE sync-wait per instruction
    (setupSyncWait: 'Too many sync wait commands') while Tile emits several.
    Hoist all-but-one wait onto standalone EventSemaphore instructions
    placed immediately before, on the same engine — semantically identical
    (sequencer stalls at each wait in order)."""
    for f in nc.m.functions:
        for blk in f.blocks:
            insts = list(blk.instructions)
            out, changed = [], False
            for inst in insts:
                si = inst.sync_info
                if si is not None and len(si.on_wait) > 1:
                    waits = list(si.on_wait)
                    for i, w in enumerate(waits[:-1]):
                        ev = mybir.InstEventSemaphore(
                            name=f"{inst.name}-sw{i}", ins=[], outs=[]
                        )
                        ev.engine = inst.engine
                        ev.sync_info = mybir.SyncInfo(on_wait=[w], on_update=[])
                        out.append(ev)
                    si.on_wait = [waits[-1]]
                    changed = True
                out.append(inst)
            if changed:
                blk.instructions = out


def build_program(repeat: int = 1, loop_reps: bool = False) -> bass.Bass:
    """Emit the single-core SPMD program (same program, per-core data).

    repeat > 1 python-unrolls the whole batch loop (idempotent).
    loop_reps=True instead wraps the batch loop in a hardware For_i whose
    trip count comes from an extra int32 input "reps" — used by test.py to
    time steady-state iterations with launch overhead cancelled exactly.
    """
    nc = bass.Bass()

    d_in = [
        nc.declare_dram_parameter(nm, [B_LOC, CH, 128, HW], FP16, isOutput=False)
        for nm in ("y", "x0", "x1", "x2", "x3")
    ]
    # Host-side pre-transposed / folded params (see make_in_maps):
    #   w1T[p, j, m]  = conv1_w[m, 128j + p]
    #   w2T[m, j, p]  = conv2_w[128j + p, m]
    #   c2bT[p, j]    = conv2_b[128j + p]
    #   scale_eff     = gamma / sqrt(var+eps) / HW      (means come as sums)
    #   bias_eff      = beta - mean * gamma / sqrt(var+eps)
    d_w1T = nc.declare_dram_parameter("w1T", [128, NCH, MID], FP32, isOutput=False)
    d_w2T = nc.declare_dram_parameter("w2T", [MID, NCH, 128], FP32, isOutput=False)
    d_c2bT = nc.declare_dram_parameter("c2bT", [128, NCH], FP32, isOutput=False)
    d_scale = nc.declare_dram_parameter("scale_eff", [MID, 1], FP32, isOutput=False)
    d_bias = nc.declare_dram_parameter("bias_eff", [MID, 1], FP32, isOutput=False)
    d_ident = nc.declare_dram_parameter("ident", [128, 128], FP16, isOutput=False)
    d_out = nc.declare_dram_parameter("out", [B_LOC, CH, 128, HW], FP16, isOutput=True)
    d_reps = (
        nc.declare_dram_parameter("reps", [1, 1], mybir.dt.int32, isOutput=False)
        if loop_reps
        else None
    )

    with tile.TileContext(nc) as tc, ExitStack() as ctx:
        cpool = ctx.enter_context(tc.tile_pool(name="cpool", bufs=1))
        ppool = ctx.enter_context(tc.tile_pool(name="ppool", bufs=2, space="PSUM"))
        dpool = ctx.enter_context(tc.tile_pool(name="dpool", bufs=2))
        spool = ctx.enter_context(tc.tile_pool(name="spool", bufs=2))

        # ---------------- parameter prep (once) ----------------
        # Params arrive pre-transposed from the host; matmul stationary
        # tensors are "laundered" through one DVE copy each so PE matmuls
        # (which tolerate only ONE sync-wait on their embedded weight load)
        # depend on a single producer proc (DVE).
        w1s = cpool.tile([128, NCH, MID], FP32, name="w1s", tag="w1s")
        w1T = cpool.tile([128, NCH, MID], FP32, name="w1T", tag="w1T")
        nc.sync.dma_start(out=w1s[:], in_=d_w1T[:])
        nc.vector.tensor_copy(w1T[:], w1s[:])

        w2s = cpool.tile([MID, NCH, 128], FP32, name="w2s", tag="w2s")
        w2T = cpool.tile([MID, NCH, 128], FP32, name="w2T", tag="w2T")
        nc.sync.dma_start(out=w2s[:], in_=d_w2T[:])
        nc.vector.tensor_copy(w2T[:], w2s[:])

        c2bT = cpool.tile([128, NCH], FP32, name="c2bT", tag="c2bT")
        nc.sync.dma_start(out=c2bT[:], in_=d_c2bT[:])
        scale_eff = cpool.tile([MID, 1], FP32, name="scale_eff", tag="scale_eff")
        nc.sync.dma_start(out=scale_eff[:], in_=d_scale[:])
        bias_eff = cpool.tile([MID, 1], FP32, name="bias_eff", tag="bias_eff")
        nc.sync.dma_start(out=bias_eff[:], in_=d_bias[:])

        idents = cpool.tile([128, 128], FP16, name="idents", tag="idents")
        ident = cpool.tile([128, 128], FP16, name="ident", tag="ident")
        nc.sync.dma_start(out=idents[:], in_=d_ident[:])
        nc.vector.tensor_copy(ident[:], idents[:])

        def batch_body(b: int) -> None:
            # Load the 5 feature maps for this batch: [128, ch, hw] fp16.
            tiles = []
            for t in range(NT):
                dt_ = dpool.tile(
                    [128, CH, HW], FP16, name=f"d{t}", tag=f"d{t}", bufs=3
                )
                nc.sync.dma_start(
                    out=dt_[:], in_=d_in[t][b].rearrange("c p f -> p c f")
                )
                tiles.append(dt_)

            # Channel sums -> mean_t[:, j], j = t*CH + ch. DVE tensor_scalar
            # in 4x fp16 mode with an fp32 accumulator output.
            mean_t = spool.tile([128, NCH], FP32, name="mean_t", tag="mean_t", bufs=2)
            scr = spool.tile([128, HW], FP16, name="scr", tag="scr", bufs=2)
            for t in range(NT):
                for ch in range(CH):
                    j = t * CH + ch
                    nc.vector.tensor_scalar(
                        out=scr[:],
                        in0=tiles[t][:, ch, :],
                        scalar1=1.0,
                        scalar2=None,
                        op0=ALU.mult,
                        op1=ALU.add,
                        accum_out=mean_t[:, j : j + 1],
                    )

            # Gate MLP on PE: h_raw[mid] = sum_j w1T[:,j,:].T @ sums[:,j]
            hps = ppool.tile([MID, 1], FP32, name="hps", tag="hps")
            for j in range(NCH):
                nc.tensor.matmul(
                    hps[:],
                    w1T[:, j, :],
                    mean_t[:, j : j + 1],
                    start=(j == 0),
                    stop=(j == NCH - 1),
                )
            h_sb = spool.tile([MID, 1], FP32, name="h_sb", tag="h_sb", bufs=2)
            nc.scalar.activation(
                out=h_sb[:], in_=hps[:], func=AF.Relu,
                bias=bias_eff[:], scale=scale_eff[:],
            )
            # Logits (pre-bias), transposed into channel-on-partition layout:
            # wps[p, j] = w[128j + p] - conv2_b[128j + p]
            wps = ppool.tile([128, NCH], FP32, name="wps", tag="wps")
            for j in range(NCH):
                nc.tensor.matmul(
                    wps[:, j : j + 1], w2T[:, j, :], h_sb[:], start=True, stop=True
                )

            # Gates: cols 0..1 = sigmoid self gate; cols 2..9 = exp for softmax.
            gat = spool.tile([128, NCH], FP32, name="gat", tag="gat", bufs=2)
            for ch in range(CH):
                nc.scalar.activation(
                    out=gat[:, ch : ch + 1], in_=wps[:, ch : ch + 1],
                    func=AF.Sigmoid, bias=c2bT[:, ch : ch + 1],
                )
            for j in range(CH, NCH):
                nc.scalar.activation(
                    out=gat[:, j : j + 1], in_=wps[:, j : j + 1],
                    func=AF.Exp, bias=c2bT[:, j : j + 1],
                )
            # softmax over k: columns 2+2k+ch, k=0..3.
            gk = gat[:, CH:NCH].rearrange("p (k c) -> p c k", c=CH)
            esum = spool.tile([128, CH, 1], FP32, name="esum", tag="esum", bufs=2)
            nc.vector.reduce_sum(out=esum[:], in_=gk, axis=mybir.AxisListType.X)
            rinv = spool.tile([128, CH, 1], FP32, name="rinv", tag="rinv", bufs=2)
            nc.vector.reciprocal(rinv[:], esum[:])
            for ch in range(CH):
                nc.vector.tensor_scalar_mul(
                    out=gk[:, ch, :], in0=gk[:, ch, :], scalar1=rinv[:, ch, :]
                )

            # Gate diagonals diag[p, j, q] = g_j[p] * I[p, q] (fp16), j = t*CH+ch
            # (t=0 is the sigmoid self gate), so pass 2 runs fully on PE.
            diag = spool.tile([128, NT * CH, 128], FP16, name="diag", tag="diag", bufs=2)
            for j in range(NCH):
                nc.vector.tensor_scalar_mul(
                    out=diag[:, j, :],
                    in0=ident[:],
                    scalar1=gat[:, j : j + 1],
                )

            # Pass 2 on PE: psum = diag(w1) @ y + sum_k diag(g_k) @ x_k per
            # 512-wide half; merge is a pure PSUM->SBUF fp16 downcast copy,
            # alternated between ACT and DVE (Pool cannot access PSUM).
            acc = dpool.tile([128, CH, HW], FP16, name="acc", tag="acc", bufs=2)
            for ch in range(CH):
                for h2 in range(2):
                    sl = slice(h2 * HWH, (h2 + 1) * HWH)
                    ps = ppool.tile([128, HWH], FP32, name="ps", tag="ps", bufs=4)
                    for t in range(NT):
                        nc.tensor.matmul(
                            ps[:],
                            diag[:, t * CH + ch, :],
                            tiles[t][:, ch, sl],
                            start=(t == 0),
                            stop=(t == NT - 1),
                        )
                    if (ch * 2 + h2) % 2 == 0:
                        nc.vector.tensor_copy(acc[:, ch, sl], ps[:])
                    else:
                        nc.scalar.copy(acc[:, ch, sl], ps[:])
            nc.sync.dma_start(out=d_out[b].rearrange("c p f -> p c f"), in_=acc[:])

        # ---------------- main loop over local batches ----------------
        if loop_reps:
            reps_sb = cpool.tile([1, 1], mybir.dt.int32, name="reps_sb", tag="reps_sb")
            nc.sync.dma_start(out=reps_sb[:], in_=d_reps[:])
            reps_val = nc.values_load(
                reps_sb[0:1, 0:1],
                min_val=1,
                max_val=1_000_000,
                skip_runtime_bounds_check=True,
            )
            with tc.For_i(0, reps_val):
                for b in range(B_LOC):
                    batch_body(b)
        else:
            for b in [i % B_LOC for i in range(B_LOC * repeat)]:
                batch_body(b)

    _split_waits(nc)
    return nc


_CACHE: dict = {}


def _get_program() -> bass.Bass:
    if "nc" not in _CACHE:
        _CACHE["nc"] = build_program()
    return _CACHE["nc"]


def make_in_maps(inputs: dict, reps: int | None = None) -> list:
    """Shard full inputs into per-core input maps (batch-parallel)."""
    f32 = lambda a: np.ascontiguousarray(np.asarray(a), dtype=np.float32)
    f16 = lambda a: np.ascontiguousarray(
        np.asarray(a, dtype=np.float32).astype(np.float16)
    )
    y = f16(inputs["y"]).reshape(B, CH, 128, HW)
    xs = [f16(inputs[f"x{k}"]).reshape(B, CH, 128, HW) for k in range(K)]

    conv1_w = f32(inputs["conv1_w"])               # [MID, FEAT]
    conv2_w = f32(inputs["conv2_w"])               # [FEAT, MID]
    conv2_b = f32(inputs["conv2_b"])               # [FEAT]
    gamma = f32(inputs["bn_gamma"]).reshape(MID)
    beta = f32(inputs["bn_beta"]).reshape(MID)
    mean = f32(inputs["bn_mean"]).reshape(MID)
    var = f32(inputs["bn_var"]).reshape(MID)

    s_bn = gamma / np.sqrt(var + EPS)
    shared = {
        "w1T": np.ascontiguousarray(
            conv1_w.reshape(MID, NCH, 128).transpose(2, 1, 0)
        ),
        "w2T": np.ascontiguousarray(
            conv2_w.reshape(NCH, 128, MID).transpose(2, 0, 1)
        ),
        "c2bT": np.ascontiguousarray(conv2_b.reshape(NCH, 128).T),
        "scale_eff": np.ascontiguousarray((s_bn / HW).reshape(MID, 1)),
        "bias_eff": np.ascontiguousarray((beta - mean * s_bn).reshape(MID, 1)),
        "ident": np.eye(128, dtype=np.float16),
    }
    if reps is not None:
        shared["reps"] = np.full((1, 1), reps, dtype=np.int32)
    in_maps = []
    for core in range(N_CORES):
        sl = slice(core * B_LOC, (core + 1) * B_LOC)
        m = {"y": np.ascontiguousarray(y[sl])}
        for k in range(K):
            m[f"x{k}"] = np.ascontiguousarray(xs[k][sl])
        m.update(shared)
        in_maps.append(m)
    return in_maps


def kernel(**inputs) -> np.ndarray:
    nc = _get_program()
    in_maps = make_in_maps(inputs)
    res = run_bass_kernel_spmd(nc, in_maps, list(range(N_CORES)))
    _CACHE["last_results"] = res
    out = np.concatenate(
        [res.results[i]["out"].reshape(B_LOC, C, H, W) for i in range(N_CORES)],
        axis=0,
    )
    return out.astype(np.float32)


# revision 12
# speedup vs baseline: 1.0233x; 1.0233x over previous
"""Trainium2 Bass kernel for nn_GatedJunction (gated multi-branch junction).

Math (per batch element b):
    m_y  = mean_hw(y[b])                     # [C]
    m_xk = mean_hw(x_k[b])                   # [C] for k=0..3
    feats = concat(m_y, m_x0..m_x3)          # [5C] = [1280]
    h  = relu(bn(feats @ conv1_w.T))         # [32]
    w  = h @ conv2_w.T + conv2_b             # [1280] -> [5, 256]
    w1 = sigmoid(w[0])                       # self gate  [256]
    w2 = softmax_k(w[1:])                    # branch gates [4, 256]
    out[b] = y[b]*w1[:,None,None] + sum_k w2[k][:,None,None]*x_k[b]

Sharding: data-parallel over batch. 8 cores x 4 batch elements each.
Params are tiny, pre-transposed/folded on the host, replicated per core.

This version halves HBM traffic vs fp32 by moving the 5 big input maps and
the output to fp16 (quantization error ~1e-3 rel, well inside tolerance):
  - inputs are cast to fp16 on the host; tiles are [128, 2, 1024] fp16
  - channel sums for the gate MLP: tensor_scalar(accum_out=...) split
    DVE/ACT; the DVE ones run in the 4x fp16 perf mode
  - gate MLP runs on PE in fp32 (tiny), gates land channel-on-partition
  - pass 2 = y*w1 + sum_k g_k*x_k runs as fp16 tensor_scalar products
    (DVE 4x mode / Pool / ACT copy-scale) combined by fp16 tensor_tensor
    adds (DVE 2x mode) and a fused Pool scalar_tensor_tensor
  - the batch loop is software-pipelined one deep: pass 2 of batch b-1 is
    emitted between the gate MLP of batch b and its own consumers, so DVE
    never idles waiting on the PE gate-MLP latency
(A previous attempt ran pass 2 on PE as diag(g) matmuls: PE never ramps
out of its low p-state on this bursty stream, costing ~80us/iter. Avoid.)
"""

import sys

for _p in ("/root/.axon_site/_ro/trn_rl_repo", "/opt/trn_rl_repo"):
    if _p not in sys.path:
        sys.path.append(_p)

from contextlib import ExitStack

import numpy as np

import concourse.bass as bass
import concourse.tile as tile
from concourse import mybir
from concourse.bass_utils import run_bass_kernel_spmd

# Problem constants (hardcoded from the spec).
B, K, C, H, W = 32, 4, 256, 32, 32
MID = 32
EPS = 1e-5
HW = H * W          # 1024
HWH = HW // 2       # 512 (one PSUM bank of fp32)
N_CORES = 8
B_LOC = B // N_CORES  # 4
NT = K + 1          # 5 tensors: y, x0..x3
FEAT = NT * C       # 1280
NCH = FEAT // 128   # 10 feature chunks of 128
CH = C // 128       # 2 channel chunks per tensor

FP32 = mybir.dt.float32
FP16 = mybir.dt.float16
ALU = mybir.AluOpType
AF = mybir.ActivationFunctionType


def _split_waits(nc: bass.Bass) -> None:
    """This toolchain's walrus accepts only ONE sync-wait per instruction
    (setupSyncWait: 'Too many sync wait commands') while Tile emits several.
    Hoist all-but-one wait onto standalone EventSemaphore instructions
    placed immediately before, on the same engine — semantically identical
    (sequencer stalls at each wait in order)."""
    for f in nc.m.functions:
        for blk in f.blocks:
            insts = list(blk.instructions)
            out, changed = [], False
            for inst in insts:
                si = inst.sync_info
                if si is not None and len(si.on_wait) > 1:
                    waits = list(si.on_wait)
                    for i, w in enumerate(waits[:-1]):
                        ev = mybir.InstEventSemaphore(
                            name=f"{inst.name}-sw{i}", ins=[], outs=[]
                        )
                        ev.engine = inst.engine
                        ev.sync_info = mybir.SyncInfo(on_wait=[w], on_update=[])
                        out.append(ev)
                    si.on_wait = [waits[-1]]
                    changed = True
                out.append(inst)
            if changed:
                blk.instructions = out


def build_program(repeat: int = 1, loop_reps: bool = False) -> bass.Bass:
    """Emit the single-core SPMD program (same program, per-core data).

    repeat > 1 python-unrolls the whole batch loop (idempotent).
    loop_reps=True instead wraps the batch loop in a hardware For_i whose
    trip count comes from an extra int32 input "reps" — used by test.py to
    time steady-state iterations with launch overhead cancelled exactly.
    """
    nc = bass.Bass()

    d_in = [
        nc.declare_dram_parameter(nm, [B_LOC, CH, 128, HW], FP16, isOutput=False)
        for nm in ("y", "x0", "x1", "x2", "x3")
    ]
    # Host-side pre-transposed / folded params (see make_in_maps):
    #   w1T[p, j, m]  = conv1_w[m, 128j + p]
    #   w2T[m, j, p]  = conv2_w[128j + p, m]
    #   c2bT[p, j]    = conv2_b[128j + p]
    #   scale_eff     = gamma / sqrt(var+eps) / HW      (means come as sums)
    #   bias_eff      = beta - mean * gamma / sqrt(var+eps)
    d_w1T = nc.declare_dram_parameter("w1T", [128, NCH, MID], FP32, isOutput=False)
    d_w2T = nc.declare_dram_parameter("w2T", [MID, NCH, 128], FP32, isOutput=False)
    d_c2bT = nc.declare_dram_parameter("c2bT", [128, NCH], FP32, isOutput=False)
    d_scale = nc.declare_dram_parameter("scale_eff", [MID, 1], FP32, isOutput=False)
    d_bias = nc.declare_dram_parameter("bias_eff", [MID, 1], FP32, isOutput=False)
    d_out = nc.declare_dram_parameter("out", [B_LOC, CH, 128, HW], FP16, isOutput=True)
    d_reps = (
        nc.declare_dram_parameter("reps", [1, 1], mybir.dt.int32, isOutput=False)
        if loop_reps
        else None
    )

    with tile.TileContext(nc) as tc, ExitStack() as ctx:
        cpool = ctx.enter_context(tc.tile_pool(name="cpool", bufs=1))
        ppool = ctx.enter_context(tc.tile_pool(name="ppool", bufs=2, space="PSUM"))
        dpool = ctx.enter_context(tc.tile_pool(name="dpool", bufs=2))
        spool = ctx.enter_context(tc.tile_pool(name="spool", bufs=2))

        # ---------------- parameter prep (once) ----------------
        # Params arrive pre-transposed from the host; matmul stationary
        # tensors are "laundered" through one DVE copy each so PE matmuls
        # (which tolerate only ONE sync-wait on their embedded weight load)
        # depend on a single producer proc (DVE).
        w1s = cpool.tile([128, NCH, MID], FP32, name="w1s", tag="w1s")
        w1T = cpool.tile([128, NCH, MID], FP32, name="w1T", tag="w1T")
        nc.sync.dma_start(out=w1s[:], in_=d_w1T[:])
        nc.vector.tensor_copy(w1T[:], w1s[:])

        w2s = cpool.tile([MID, NCH, 128], FP32, name="w2s", tag="w2s")
        w2T = cpool.tile([MID, NCH, 128], FP32, name="w2T", tag="w2T")
        nc.sync.dma_start(out=w2s[:], in_=d_w2T[:])
        nc.vector.tensor_copy(w2T[:], w2s[:])

        c2bT = cpool.tile([128, NCH], FP32, name="c2bT", tag="c2bT")
        nc.sync.dma_start(out=c2bT[:], in_=d_c2bT[:])
        scale_eff = cpool.tile([MID, 1], FP32, name="scale_eff", tag="scale_eff")
        nc.sync.dma_start(out=scale_eff[:], in_=d_scale[:])
        bias_eff = cpool.tile([MID, 1], FP32, name="bias_eff", tag="bias_eff")
        nc.sync.dma_start(out=bias_eff[:], in_=d_bias[:])

        def batch_front_a(b: int):
            """Loads + channel sums + PE conv1 for batch b (nothing here
            waits on the gate MLP, so ACT/DVE queues stay stall-free)."""
            tiles = []
            for t in range(NT):
                dt_ = dpool.tile(
                    [128, CH, HW], FP16, name=f"d{t}", tag=f"d{t}", bufs=3
                )
                nc.sync.dma_start(
                    out=dt_[:], in_=d_in[t][b].rearrange("c p f -> p c f")
                )
                tiles.append(dt_)

            # Channel sums -> mean_t[:, j], j = t*CH + ch, fp32 accumulators.
            # Split 3 on ACT (earliest-loaded tiles) / 7 on DVE (4x fp16 TS).
            mean_t = spool.tile([128, NCH], FP32, name="mean_t", tag="mean_t", bufs=2)
            scr = spool.tile([128, HW], FP16, name="scr", tag="scr", bufs=2)
            scr_a = spool.tile([128, HW], FP16, name="scr_a", tag="scr_a", bufs=2)
            for j in range(NCH):
                t, ch = divmod(j, CH)
                if j < 3:
                    nc.scalar.activation(
                        out=scr_a[:],
                        in_=tiles[t][:, ch, :],
                        func=AF.Copy,
                        accum_out=mean_t[:, j : j + 1],
                    )
                else:
                    nc.vector.tensor_scalar(
                        out=scr[:],
                        in0=tiles[t][:, ch, :],
                        scalar1=1.0,
                        scalar2=None,
                        op0=ALU.mult,
                        op1=ALU.add,
                        accum_out=mean_t[:, j : j + 1],
                    )

            # Gate MLP layer 1 on PE: h_raw[m] = sum_j w1T[:,j,:].T @ sums[:,j]
            hps = ppool.tile([MID, 1], FP32, name="hps", tag="hps")
            for j in range(NCH):
                nc.tensor.matmul(
                    hps[:],
                    w1T[:, j, :],
                    mean_t[:, j : j + 1],
                    start=(j == 0),
                    stop=(j == NCH - 1),
                )
            return tiles, hps

        def batch_front_b(hps):
            """Everything downstream of the PE conv1 result: relu, conv2,
            bias, sigmoid/exp, softmax. Returns the gate tile."""
            h_sb = spool.tile([MID, 1], FP32, name="h_sb", tag="h_sb", bufs=2)
            nc.scalar.activation(
                out=h_sb[:], in_=hps[:], func=AF.Relu,
                bias=bias_eff[:], scale=scale_eff[:],
            )
            # Logits (pre-bias), transposed into channel-on-partition layout:
            # wps[p, j] = w[128j + p] - conv2_b[128j + p]
            wps = ppool.tile([128, NCH], FP32, name="wps", tag="wps")
            for j in range(NCH):
                nc.tensor.matmul(
                    wps[:, j : j + 1], w2T[:, j, :], h_sb[:], start=True, stop=True
                )

            # Bias in one DVE add, then batched sigmoid [.,2] / exp [.,8].
            wlog = spool.tile([128, NCH], FP32, name="wlog", tag="wlog", bufs=2)
            nc.vector.tensor_tensor(out=wlog[:], in0=wps[:], in1=c2bT[:], op=ALU.add)
            gat = spool.tile([128, NCH], FP32, name="gat", tag="gat", bufs=2)
            nc.scalar.activation(
                out=gat[:, 0:CH], in_=wlog[:, 0:CH], func=AF.Sigmoid
            )
            nc.scalar.activation(
                out=gat[:, CH:NCH], in_=wlog[:, CH:NCH], func=AF.Exp
            )
            # softmax over k: columns 2+2k+ch, k=0..3.
            gk = gat[:, CH:NCH].rearrange("p (k c) -> p c k", c=CH)
            esum = spool.tile([128, CH, 1], FP32, name="esum", tag="esum", bufs=2)
            nc.vector.reduce_sum(out=esum[:], in_=gk, axis=mybir.AxisListType.X)
            rinv = spool.tile([128, CH, 1], FP32, name="rinv", tag="rinv", bufs=2)
            nc.vector.reciprocal(rinv[:], esum[:])
            for ch in range(CH):
                nc.vector.tensor_scalar_mul(
                    out=gk[:, ch, :], in0=gk[:, ch, :], scalar1=rinv[:, ch, :]
                )
            return gat

        def batch_back(b: int, tiles, gat):
            """Pass 2 for batch b: out = y*w1 + sum_k g_k*x_k, all fp16
            elementwise. Products: 3 on DVE (4x TS) + 2 on ACT (copy-scale);
            adds: 3 on DVE (2x TT) + the final one on Pool (plain TT add —
            Pool rejects pointer-scalar tensor_scalar/STT at codegen)."""

            def g(t, ch):  # gate column for tensor t, channel chunk ch
                return gat[:, t * CH + ch : t * CH + ch + 1]

            acc = dpool.tile([128, CH, HW], FP16, name="acc", tag="acc", bufs=2)
            pa = spool.tile([128, HW], FP16, name="pa", tag="pa", bufs=2)
            pb = spool.tile([128, HW], FP16, name="pb", tag="pb", bufs=2)
            pc = spool.tile([128, HW], FP16, name="pc", tag="pc", bufs=2)
            pd = spool.tile([128, HW], FP16, name="pd", tag="pd", bufs=2)
            sab = spool.tile([128, HW], FP16, name="sab", tag="sab", bufs=2)
            scd = spool.tile([128, HW], FP16, name="scd", tag="scd", bufs=2)
            sabcd = spool.tile([128, HW], FP16, name="sabcd", tag="sabcd", bufs=2)
            pe_ = spool.tile([128, HW], FP16, name="pe", tag="pe", bufs=2)
            for ch in range(CH):
                # DVE products (4x fp16 TS): y*w1, x0*g0, x2*g2
                nc.vector.tensor_scalar_mul(
                    out=pa[:], in0=tiles[0][:, ch, :], scalar1=g(0, ch)
                )
                nc.vector.tensor_scalar_mul(
                    out=pb[:], in0=tiles[1][:, ch, :], scalar1=g(1, ch)
                )
                # ACT products (copy with per-partition scale): x1*g1, x3*g3
                nc.scalar.activation(
                    out=pc[:], in_=tiles[2][:, ch, :], func=AF.Copy,
                    scale=g(2, ch),
                )
                nc.vector.tensor_scalar_mul(
                    out=pd[:], in0=tiles[3][:, ch, :], scalar1=g(3, ch)
                )
                nc.scalar.activation(
                    out=pe_[:], in_=tiles[4][:, ch, :], func=AF.Copy,
                    scale=g(4, ch),
                )
                # Adds: (pa+pb), (pc+pd) and their sum on DVE; final on Pool
                nc.vector.tensor_tensor(
                    out=sab[:], in0=pa[:], in1=pb[:], op=ALU.add
                )
                nc.vector.tensor_tensor(
                    out=scd[:], in0=pc[:], in1=pd[:], op=ALU.add
                )
                nc.vector.tensor_tensor(
                    out=sabcd[:], in0=sab[:], in1=scd[:], op=ALU.add
                )
                nc.gpsimd.tensor_tensor(
                    out=acc[:, ch, :], in0=sabcd[:], in1=pe_[:], op=ALU.add
                )
            nc.sync.dma_start(out=d_out[b].rearrange("c p f -> p c f"), in_=acc[:])

        def batch_seq():
            """Software pipeline: pass 2 of batch b-1 is emitted between
            conv1 of batch b and the MLP-dependent tail of batch b, so the
            in-order ACT/DVE queues never park behind PE-gated work."""
            prev = None
            for b in range(B_LOC):
                tiles, hps = batch_front_a(b)
                if prev is not None:
                    batch_back(prev[0], prev[1], prev[2])
                gat = batch_front_b(hps)
                prev = (b, tiles, gat)
            batch_back(prev[0], prev[1], prev[2])

        # ---------------- main loop over local batches ----------------
        if loop_reps:
            reps_sb = cpool.tile([1, 1], mybir.dt.int32, name="reps_sb", tag="reps_sb")
            nc.sync.dma_start(out=reps_sb[:], in_=d_reps[:])
            reps_val = nc.values_load(
                reps_sb[0:1, 0:1],
                min_val=1,
                max_val=1_000_000,
                skip_runtime_bounds_check=True,
            )
            with tc.For_i(0, reps_val):
                batch_seq()
        else:
            for _ in range(repeat):
                batch_seq()

    _split_waits(nc)
    return nc


_CACHE: dict = {}


def _get_program() -> bass.Bass:
    if "nc" not in _CACHE:
        _CACHE["nc"] = build_program()
    return _CACHE["nc"]


def make_in_maps(inputs: dict, reps: int | None = None) -> list:
    """Shard full inputs into per-core input maps (batch-parallel)."""
    f32 = lambda a: np.ascontiguousarray(np.asarray(a), dtype=np.float32)
    f16 = lambda a: np.ascontiguousarray(
        np.asarray(a, dtype=np.float32).astype(np.float16)
    )
    y = f16(inputs["y"]).reshape(B, CH, 128, HW)
    xs = [f16(inputs[f"x{k}"]).reshape(B, CH, 128, HW) for k in range(K)]

    conv1_w = f32(inputs["conv1_w"])               # [MID, FEAT]
    conv2_w = f32(inputs["conv2_w"])               # [FEAT, MID]
    conv2_b = f32(inputs["conv2_b"])               # [FEAT]
    gamma = f32(inputs["bn_gamma"]).reshape(MID)
    beta = f32(inputs["bn_beta"]).reshape(MID)
    mean = f32(inputs["bn_mean"]).reshape(MID)
    var = f32(inputs["bn_var"]).reshape(MID)

    s_bn = gamma / np.sqrt(var + EPS)
    shared = {
        "w1T": np.ascontiguousarray(
            conv1_w.reshape(MID, NCH, 128).transpose(2, 1, 0)
        ),
        "w2T": np.ascontiguousarray(
            conv2_w.reshape(NCH, 128, MID).transpose(2, 0, 1)
        ),
        "c2bT": np.ascontiguousarray(conv2_b.reshape(NCH, 128).T),
        "scale_eff": np.ascontiguousarray((s_bn / HW).reshape(MID, 1)),
        "bias_eff": np.ascontiguousarray((beta - mean * s_bn).reshape(MID, 1)),
    }
    if reps is not None:
        shared["reps"] = np.full((1, 1), reps, dtype=np.int32)
    in_maps = []
    for core in range(N_CORES):
        sl = slice(core * B_LOC, (core + 1) * B_LOC)
        m = {"y": np.ascontiguousarray(y[sl])}
        for k in range(K):
            m[f"x{k}"] = np.ascontiguousarray(xs[k][sl])
        m.update(shared)
        in_maps.append(m)
    return in_maps


def kernel(**inputs) -> np.ndarray:
    nc = _get_program()
    in_maps = make_in_maps(inputs)
    res = run_bass_kernel_spmd(nc, in_maps, list(range(N_CORES)))
    _CACHE["last_results"] = res
    out = np.concatenate(
        [res.results[i]["out"].reshape(B_LOC, C, H, W) for i in range(N_CORES)],
        axis=0,
    )
    return out.astype(np.float32)
